# revision 15
# baseline (speedup 1.0000x reference)
"""Bahdanau attention on 8 Trainium2 NeuronCores.

Data-parallel over the batch axis B=16 (2 batches per core), params
replicated.  Per core, for its 2 batches b and D=2 decoder states d:

  qh[r,:]   = ht[d,b] @ q_w.T + q_b                  (r = b*2+d)
  kh[h,s]   = sum_e k_w.T[e,h] * hs[b].T[e,s]        (f32r matmul, PE)
  T[h,s]    = tanh(kh[h,s] + qh[r,h] + k_b[h])       (ACT, per-partition bias)
  e[r,s]    = sum_h v[h] * T[h,s]                    (M=1 f32r matmul, PSUM accum)
  at[r,:]   = softmax(e[r,:])                        (v_b dropped: softmax shift-invariant)
  ctx[r,g]  = sum_s at.T[s,r] * hs[b][s,g]           (f32r matmul)
  c[r,o]    = sum_g ctx.T[g,r] * c_w.T[g,o] + c_b[o]

All matmuls use float32r (TF32-like, 1 cyc/row at N>=256; measured ~1.3e-4
rel err on HW vs 4 cyc/row for exact fp32).  hs is shipped in both [s,e]
and transposed [e,s] layouts so both the kh and ctx contractions run with
the contracted axis on partitions.
"""

import os
import sys
import time

for _p in ("/opt/trn_rl_repo", os.path.expanduser("~/.axon_site/_ro/trn_rl_repo")):
    if os.path.isdir(_p) and _p not in sys.path:
        sys.path.insert(0, _p)

import numpy as np

import concourse.bass as bass
import concourse.mybir as mybir
import concourse.tile as tile
from concourse.masks import make_identity
from concourse.vector_clock import ScopedClock

# ---------------------------------------------------------------------------
# Workaround: this container's walrus build rejects more than one sync-wait
# command per instruction (setupSyncWait in CoreV3GenImpl).  Hoist extra
# waits onto preceding single-wait same-engine nops.
# ---------------------------------------------------------------------------

_MAX_WAITS = 1


def _patched_drain_and_barrier(self, tick_clock, wait_clock):
    nc = self.nc
    probe = nc.sync.nop(nofuse=True)
    wait_clock.add_sem_waits(probe.ins, ScopedClock({None: tick_clock.global_clock}))
    si = probe.ins.sync_info
    waits = list(si.on_wait) if si is not None and si.on_wait else []
    upds = list(si.on_update) if si is not None and si.on_update else []
    probe.ins.sync_info = mybir.SyncInfo(on_wait=waits[:1], on_update=upds)
    for w in waits[1:]:
        n2 = nc.sync.nop(nofuse=True)
        n2.ins.sync_info = mybir.SyncInfo(on_wait=[w], on_update=[])
    nc.sync.drain()
    nc.all_engine_barrier()
    assert self.sems is not None
    popped = nc._tile_sem_poison_stack.pop()
    assert popped is self._sem_poison
    nc.clear_and_free_semaphores(list(self.sems.allocated().values()))
    nc.all_engine_barrier()


_orig_commit = tile.TileContext._commit_instruction


def _is_dma(inst):
    return type(inst).__name__ in ("InstDMACopy", "InstTensorLoad", "InstTensorSave")


def _patched_commit(self, inst, lazy_reg_writes=True):
    si = getattr(inst, "sync_info", None)
    if (
        si is not None
        and si.on_wait
        and len(si.on_wait) > _MAX_WAITS
        and type(inst).__name__.startswith("Inst")
        and inst.engine != mybir.EngineType.Unassigned
    ):
        waits = list(si.on_wait)
        if _is_dma(inst):
            # Keep the DMA's own HW-queue wait on the descriptor; hoist the
            # compute-engine WAR wait onto the issuing sequencer (SP pushes
            # descriptors in order, so stalling it first is sound).
            keep_idx = next(
                (i for i, w in enumerate(waits)
                 if str(w.ant_name).startswith("DMAHW")), 0,
            )
            waits = [waits[keep_idx]] + [
                w for i, w in enumerate(waits) if i != keep_idx
            ]
        upds = list(si.on_update) if si.on_update else []
        inst.sync_info = mybir.SyncInfo(on_wait=waits[:_MAX_WAITS], on_update=upds)
        for w in waits[_MAX_WAITS:]:
            nop = mybir.InstNoOp(
                name=self.nc.get_next_instruction_name(),
                engine=inst.engine,
                sync_info=mybir.SyncInfo(on_wait=[w], on_update=[]),
                bass_nofuse=True,
            )
            _orig_commit(self, nop, lazy_reg_writes=False)
    return _orig_commit(self, inst, lazy_reg_writes)


def _install_patches():
    tile.TileContext._drain_and_barrier = _patched_drain_and_barrier
    tile.TileContext._commit_instruction = _patched_commit


_install_patches()

# ---------------------------------------------------------------------------
# Problem shapes (hardcoded per the grading contract)
# ---------------------------------------------------------------------------

D, B, S, DH, EH = 2, 16, 2048, 1024, 1024
NCORES = 8
BL = B // NCORES          # local batches per core
R = D * BL                # local (b, d) rows
PT = 128                  # partition tile
ET = EH // PT             # 8 contraction tiles over e/h/g
ST = S // PT              # 16 s partition-tiles
SBLK = 512                # kh s-block (one PSUM bank of fp32)
NSB = S // SBLK
OBLK = 512                # output column block for qh/ctx/c

F32 = mybir.dt.float32
F32R = mybir.dt.float32r


def build_nc(loop_n: int = 1) -> bass.Bass:
    nc = bass.Bass("TRN2", target_bir_lowering=False, debug=False, num_devices=NCORES)

    hsT = nc.declare_dram_parameter("hsT", [BL, EH, S], F32, isOutput=False)
    hs = nc.declare_dram_parameter("hs", [BL, S, EH], F32, isOutput=False)
    kwT = nc.declare_dram_parameter("kwT", [EH, DH], F32, isOutput=False)
    qwT = nc.declare_dram_parameter("qwT", [DH, DH], F32, isOutput=False)
    cwT = nc.declare_dram_parameter("cwT", [EH, DH], F32, isOutput=False)
    htT = nc.declare_dram_parameter("htT", [DH, R], F32, isOutput=False)
    qb = nc.declare_dram_parameter("qb", [1, DH], F32, isOutput=False)
    kb = nc.declare_dram_parameter("kb", [DH], F32, isOutput=False)
    cb = nc.declare_dram_parameter("cb", [1, DH], F32, isOutput=False)
    v = nc.declare_dram_parameter("v", [DH], F32, isOutput=False)
    at_o = nc.declare_dram_parameter("at_out", [D, BL, S], F32, isOutput=True)
    c_o = nc.declare_dram_parameter("c_out", [D, BL, DH], F32, isOutput=True)

    Tanh = mybir.ActivationFunctionType.Tanh
    Exp = mybir.ActivationFunctionType.Exp
    AX = mybir.AxisListType.X

    with tile.TileContext(nc) as tc:
        with (
            tc.tile_pool(name="consts", bufs=1) as consts,
            tc.tile_pool(name="bigw", bufs=1) as bigw,
            tc.tile_pool(name="hsts", bufs=2) as hsts,
            tc.tile_pool(name="hsg", bufs=2) as hsgp,
            tc.tile_pool(name="tpool", bufs=4) as tpool,
            tc.tile_pool(name="small", bufs=1) as small,
            tc.tile_pool(name="attp", bufs=2) as attp,
            tc.tile_pool(name="ps_kh", bufs=2, space="PSUM") as ps_kh,
            tc.tile_pool(name="ps_e", bufs=4, space="PSUM") as ps_e,
            tc.tile_pool(name="ps_misc", bufs=2, space="PSUM") as ps_misc,
        ):
            # ---- constants (loaded once, outside the timing loop) ----
            htT_sb = consts.tile([PT, ET, R], F32R)
            nc.sync.dma_start(
                out=htT_sb[:], in_=htT.rearrange("(t p) r -> p t r", p=PT).bitcast(F32R)
            )
            v_sb = consts.tile([PT, ET], F32R)
            nc.sync.dma_start(
                out=v_sb[:], in_=v.rearrange("(t p) -> p t", p=PT).bitcast(F32R)
            )
            qb_sb = consts.tile([PT, ET], F32)
            nc.sync.dma_start(out=qb_sb[:], in_=qb.rearrange("a (t p) -> p (a t)", p=PT))
            cb_sb = consts.tile([R, DH], F32)
            nc.gpsimd.dma_start(
                out=cb_sb[:],
                in_=cb[0:1, :].partition_broadcast(R).rearrange("p a s -> p (a s)"),
            )
            kb_sb = consts.tile([PT, ET], F32)
            nc.sync.dma_start(out=kb_sb[:], in_=kb.rearrange("(t p) -> p t", p=PT))
            # kqb[h] = k_b[h] + q_b[h]: both fold into the tanh bias
            kqb_sb = consts.tile([PT, ET], F32)
            nc.vector.tensor_add(kqb_sb[:], kb_sb[:], qb_sb[:])
            ident = consts.tile([PT, PT], F32)
            make_identity(nc, ident[:])

            def body(iv=None):
                # All weight loads live inside the body (shared slots), so a
                # timing-loop iteration does exactly one real execution's DMA.
                kwT_sb = bigw.tile([PT, ET, DH], F32R, tag="kwt")
                nc.sync.dma_start(
                    out=kwT_sb[:],
                    in_=kwT.rearrange("(t p) o -> p t o", p=PT).bitcast(F32R),
                )
                # prefetch the first kh chunk ahead of qwT so PE can start
                # as soon as kwT lands
                hsT_pre = hsts.tile([PT, ET, SBLK], F32R, tag="hsts",
                                    name="hsT_pre")
                nc.sync.dma_start(
                    out=hsT_pre[:],
                    in_=hsT[0, :, 0:SBLK]
                    .rearrange("(t p) s -> p t s", p=PT).bitcast(F32R),
                )
                qwT_sb = bigw.tile([PT, ET, DH], F32R, tag="bigw")
                nc.sync.dma_start(
                    out=qwT_sb[:],
                    in_=qwT.rearrange("(t p) o -> p t o", p=PT).bitcast(F32R),
                )

                # ---- qh = ht @ q_w.T ; bias[h, r] = qh[r, h] + k_b[h] + q_b[h] ----
                qh_sb = small.tile([R, DH], F32, tag="qh")
                for ob in range(DH // OBLK):
                    qps = ps_misc.tile([R, OBLK], F32, tag="misc")
                    for t in range(ET):
                        nc.tensor.matmul(
                            qps[:],
                            htT_sb[:, t, :],
                            qwT_sb[:, t, ob * OBLK:(ob + 1) * OBLK],
                            start=(t == 0),
                            stop=(t == ET - 1),
                        )
                    nc.vector.tensor_copy(
                        qh_sb[:, ob * OBLK:(ob + 1) * OBLK], qps[:]
                    )

                bias_sb = small.tile([PT, ET, R], F32, tag="bias")
                for t in range(ET):
                    tp = ps_misc.tile([PT, R], F32, tag="misc")
                    nc.tensor.transpose(
                        tp[:], qh_sb[:, t * PT:(t + 1) * PT], ident[:R, :R]
                    )
                    nc.vector.tensor_add(
                        bias_sb[:, t, :], tp[:], kqb_sb[:, t:t + 1].broadcast_to((PT, R))
                    )

                # cwT reuses qwT's slot: its load starts once the qh matmuls
                # release it, landing long before the c projection needs it
                cwT_sb = bigw.tile([PT, ET, DH], F32R, tag="bigw")
                nc.sync.dma_start(
                    out=cwT_sb[:],
                    in_=cwT.rearrange("(t p) o -> p t o", p=PT).bitcast(F32R),
                )

                # per-(b, d) row tensors: engines may only address partition
                # windows based at 0/32/64/96, so each row lives in its own
                # base-0 tile
                e_rows = [[small.tile([1, S], F32, tag="e", bufs=2, name=f"e_{b}_{d}")
                           for d in range(D)] for b in range(BL)]
                at_rows = [[small.tile([1, S], F32, tag="at", bufs=2, name=f"at_{b}_{d}")
                            for d in range(D)] for b in range(BL)]
                ctx_bs = [small.tile([D, DH], F32, tag="ctx", bufs=2, name=f"ctx_{b}")
                          for b in range(BL)]

                prev_vdots = []
                for b in range(BL):
                    # ---- pass A: kh, tanh, v-dot -> e rows ----
                    # The e-row PSUM accumulation group interleaves with the
                    # kh groups on PE; pin the intended PE order explicitly or
                    # the tile scheduler can deadlock on the slot/group cycle.
                    for sbi in range(NSB):
                        if b == 0 and sbi == 0:
                            hsT_t = hsT_pre
                        else:
                            hsT_t = hsts.tile([PT, ET, SBLK], F32R, tag="hsts")
                            nc.sync.dma_start(
                                out=hsT_t[:],
                                in_=hsT[b, :, sbi * SBLK:(sbi + 1) * SBLK]
                                .rearrange("(t p) s -> p t s", p=PT).bitcast(F32R),
                            )
                        eps = [
                            ps_e.tile([1, SBLK], F32, tag="e", name=f"eps{d}")
                            for d in range(D)
                        ]
                        for ot in range(ET):
                            khp = ps_kh.tile([PT, SBLK], F32, tag="kh")
                            for et in range(ET):
                                mm = nc.tensor.matmul(
                                    khp[:],
                                    kwT_sb[:, et, ot * PT:(ot + 1) * PT],
                                    hsT_t[:, et, :],
                                    start=(et == 0),
                                    stop=(et == ET - 1),
                                )
                                if et == 0:
                                    for pv in prev_vdots:
                                        tile.add_dep_helper(
                                            mm.ins, pv.ins, sync=False,
                                            reason="PE order: kh after prior vdots",
                                        )
                            prev_vdots = []
                            for d in range(D):
                                r = b * D + d
                                tt = tpool.tile([PT, SBLK], F32R, tag="T")
                                nc.scalar.activation(
                                    tt[:], khp[:], Tanh,
                                    bias=bias_sb[:, ot, r:r + 1], scale=1.0,
                                )
                                vd = nc.tensor.matmul(
                                    eps[d][:],
                                    v_sb[:, ot:ot + 1],
                                    tt[:],
                                    start=(ot == 0),
                                    stop=(ot == ET - 1),
                                )
                                prev_vdots.append(vd)
                        for d in range(D):
                            nc.vector.tensor_copy(
                                e_rows[b][d][:, sbi * SBLK:(sbi + 1) * SBLK],
                                eps[d][:],
                            )

                    # ---- softmax over s, one (b, d) row at a time ----
                    for d in range(D):
                        erow = e_rows[b][d]
                        arow = at_rows[b][d]
                        nm = small.tile([1, 1], F32, tag="sm_nm")
                        nc.vector.reduce_max(nm[:], erow[:], axis=AX, negate=True)
                        nc.scalar.activation(arow[:], erow[:], Exp, bias=nm[:],
                                             scale=1.0)
                        ssum = small.tile([1, 1], F32, tag="sm_sum")
                        nc.vector.reduce_sum(ssum[:], arow[:], axis=AX)
                        rs = small.tile([1, 1], F32, tag="sm_rs")
                        nc.vector.reciprocal(rs[:], ssum[:])
                        nc.vector.tensor_scalar_mul(arow[:], arow[:], rs[:])
                        nc.sync.dma_start(out=at_o[d, b], in_=arow[0:1, :])

                    # ---- transpose at rows -> atT [s, d] (8 per PSUM bank) ----
                    atT_sb = attp.tile([PT, ST, D], F32R, tag="atT")
                    for g4 in range(ST // 4):
                        tp8 = ps_misc.tile([PT, 4 * D], F32, tag="misc")
                        for k in range(4):
                            st = g4 * 4 + k
                            for d in range(D):
                                nc.tensor.transpose(
                                    tp8[:, k * D + d:k * D + d + 1],
                                    at_rows[b][d][:, st * PT:(st + 1) * PT],
                                    ident[:1, :1],
                                )
                        nc.vector.tensor_copy(
                            atT_sb[:, g4 * 4:(g4 + 1) * 4, :], tp8[:]
                        )

                    # ---- pass B: ctx = at @ hs ----
                    # hs[b] streams in half-height chunks (8 of 16 s-tiles) to
                    # bound SBUF; the PSUM accumulation spans both chunks.
                    for gb in range(EH // OBLK):
                        cps = ps_misc.tile([D, OBLK], F32, tag="misc")
                        for half in range(2):
                            st0 = half * (ST // 2)
                            hsg_t = hsgp.tile([PT, ST // 2, OBLK], F32R, tag="hsg")
                            nc.sync.dma_start(
                                out=hsg_t[:],
                                in_=hs[b, st0 * PT:(st0 + ST // 2) * PT,
                                       gb * OBLK:(gb + 1) * OBLK]
                                .rearrange("(t p) g -> p t g", p=PT).bitcast(F32R),
                            )
                            for sti in range(ST // 2):
                                st = st0 + sti
                                nc.tensor.matmul(
                                    cps[:],
                                    atT_sb[:, st, :],
                                    hsg_t[:, sti, :],
                                    start=(st == 0),
                                    stop=(st == ST - 1),
                                )
                        nc.vector.tensor_copy(
                            ctx_bs[b][:, gb * OBLK:(gb + 1) * OBLK], cps[:]
                        )

                # ---- c = ctx @ c_w.T + c_b ----
                ctxT_sb = small.tile([PT, ET, R], F32R, tag="ctxT")
                for b in range(BL):
                    for g4 in range(ET // 4):
                        tpc = ps_misc.tile([PT, 4 * D], F32, tag="misc")
                        for k in range(4):
                            t = g4 * 4 + k
                            nc.tensor.transpose(
                                tpc[:, k * D:(k + 1) * D],
                                ctx_bs[b][:, t * PT:(t + 1) * PT],
                                ident[:D, :D],
                            )
                        nc.vector.tensor_copy(
                            ctxT_sb[:, g4 * 4:(g4 + 1) * 4, b * D:(b + 1) * D],
                            tpc[:].rearrange("p (k d) -> p k d", d=D),
                        )
                c_sb = small.tile([R, DH], F32, tag="c")
                for ob in range(DH // OBLK):
                    cps = ps_misc.tile([R, OBLK], F32, tag="misc")
                    for t in range(ET):
                        nc.tensor.matmul(
                            cps[:],
                            ctxT_sb[:, t, :],
                            cwT_sb[:, t, ob * OBLK:(ob + 1) * OBLK],
                            start=(t == 0),
                            stop=(t == ET - 1),
                        )
                    nc.vector.tensor_add(
                        c_sb[:, ob * OBLK:(ob + 1) * OBLK], cps[:],
                        cb_sb[:, ob * OBLK:(ob + 1) * OBLK],
                    )
                for b in range(BL):
                    for d in range(D):
                        nc.sync.dma_start(out=c_o[d, b], in_=c_sb[b * D + d:b * D + d + 1, :])

            if loop_n == 1:
                body()
            else:
                with tc.For_i(0, loop_n, 1):
                    body()

    return nc


# ---------------------------------------------------------------------------
# Host-side sharding + PJRT runner (built once, cached)
# ---------------------------------------------------------------------------


def shard_inputs(hs, ht, q_w, q_b, k_w, k_b, v_w, v_b, c_w, c_b):
    """Per-core input maps.  v_b is dropped: softmax is shift-invariant, so
    it affects neither output."""
    kwT = np.ascontiguousarray(k_w.T)
    qwT = np.ascontiguousarray(q_w.T)
    cwT = np.ascontiguousarray(c_w.T)
    qb = np.ascontiguousarray(q_b[None, :])
    cb = np.ascontiguousarray(c_b[None, :])
    vv = np.ascontiguousarray(v_w[0])
    in_maps = []
    for core in range(NCORES):
        b0 = core * BL
        hs_i = np.ascontiguousarray(hs[b0:b0 + BL])
        hsT_i = np.ascontiguousarray(hs_i.transpose(0, 2, 1))
        # htT[h, r] with r = b*D + d
        htT_i = np.ascontiguousarray(
            ht[:, b0:b0 + BL, :].transpose(2, 1, 0).reshape(DH, R)
        )
        in_maps.append({
            "hsT": hsT_i, "hs": hs_i, "kwT": kwT, "qwT": qwT, "cwT": cwT,
            "htT": htT_i, "qb": qb, "kb": np.ascontiguousarray(k_b),
            "cb": cb, "v": vv,
        })
    return in_maps


class SpmdRunner:
    """Build-once, run-many PJRT executor for a Bass SPMD kernel."""

    def __init__(self, nc: bass.Bass, n_cores: int = NCORES):
        import jax
        from jax.sharding import Mesh, PartitionSpec
        from jax.experimental.shard_map import shard_map
        from concourse import bass2jax
        from concourse.bass2jax import _bass_exec_p, install_neuronx_cc_hook

        install_neuronx_cc_hook()
        self.jax = jax
        self.nc = nc
        self.n_cores = n_cores
        partition_name = (
            nc.partition_id_tensor.name if nc.partition_id_tensor else None
        )
        in_names, out_names, out_avals, zero_outs = [], [], [], []
        for alloc in nc.m.functions[0].allocations:
            if not isinstance(alloc, mybir.MemoryLocationSet):
                continue
            name = alloc.memorylocations[0].name
            if alloc.kind == "ExternalInput":
                if name != partition_name:
                    in_names.append(name)
            elif alloc.kind == "ExternalOutput":
                out_names.append(name)
                shape = tuple(alloc.tensor_shape)
                dtype = mybir.dt.np(alloc.dtype)
                out_avals.append(jax.core.ShapedArray(shape, dtype))
                zero_outs.append(np.zeros(shape, dtype))
        self.in_names, self.out_names = in_names, out_names
        self.out_avals, self.zero_outs = out_avals, zero_outs
        n_params = len(in_names)
        all_in_names = list(in_names) + list(out_names)
        if partition_name is not None:
            all_in_names.append(partition_name)

        def _body(*args):
            operands = list(args)
            if partition_name is not None:
                operands.append(bass2jax.partition_id_tensor())
            outs = _bass_exec_p.bind(
                *operands,
                out_avals=tuple(out_avals),
                in_names=tuple(all_in_names),
                out_names=tuple(out_names),
                lowering_input_output_aliases=(),
                sim_require_finite=False,
                sim_require_nnan=False,
                nc=nc,
            )
            return tuple(outs)

        devices = jax.devices()[:n_cores]
        assert len(devices) == n_cores, (
            f"need {n_cores} devices, have {len(jax.devices())}"
        )
        self.mesh = Mesh(np.asarray(devices), ("core",))
        in_specs = (PartitionSpec("core"),) * (n_params + len(out_names))
        out_specs = (PartitionSpec("core"),) * len(out_names)
        self.fn = jax.jit(
            shard_map(_body, mesh=self.mesh, in_specs=in_specs,
                      out_specs=out_specs, check_rep=False),
            keep_unused=True,
        )

    def _concat_args(self, in_maps):
        per_core = [
            [np.asarray(m[name]) for name in self.in_names] for m in in_maps
        ]
        concat_in = [
            np.concatenate([per_core[c][i] for c in range(self.n_cores)], axis=0)
            for i in range(len(self.in_names))
        ]
        concat_zeros = [
            np.zeros((self.n_cores * z.shape[0], *z.shape[1:]), z.dtype)
            for z in self.zero_outs
        ]
        return [*concat_in, *concat_zeros]

    def run(self, in_maps):
        args = self._concat_args(in_maps)
        out_arrs = self.fn(*args)
        self.jax.block_until_ready(out_arrs)
        return [
            {
                name: np.asarray(out_arrs[i]).reshape(
                    self.n_cores, *self.out_avals[i].shape
                )[c]
                for i, name in enumerate(self.out_names)
            }
            for c in range(self.n_cores)
        ]

    def time_wall(self, in_maps, iters=20):
        args = self._concat_args(in_maps)
        out = self.fn(*args)
        self.jax.block_until_ready(out)
        ts = []
        for _ in range(iters):
            t0 = time.perf_counter()
            out = self.fn(*args)
            self.jax.block_until_ready(out)
            ts.append(time.perf_counter() - t0)
        return min(ts)


_RUNNER = None


def _get_runner():
    global _RUNNER
    if _RUNNER is None:
        _RUNNER = SpmdRunner(build_nc(loop_n=1), NCORES)
    return _RUNNER


def kernel(hs, ht, q_w, q_b, k_w, k_b, v_w, v_b, c_w, c_b):
    hs = np.asarray(hs, dtype=np.float32)
    ht = np.asarray(ht, dtype=np.float32)
    in_maps = shard_inputs(
        hs, ht,
        np.asarray(q_w, np.float32), np.asarray(q_b, np.float32),
        np.asarray(k_w, np.float32), np.asarray(k_b, np.float32),
        np.asarray(v_w, np.float32), np.asarray(v_b, np.float32),
        np.asarray(c_w, np.float32), np.asarray(c_b, np.float32),
    )
    results = _get_runner().run(in_maps)
    c = np.empty((D, B, DH), np.float32)
    at = np.empty((D, B, S), np.float32)
    for core in range(NCORES):
        b0 = core * BL
        c[:, b0:b0 + BL, :] = results[core]["c_out"]
        at[:, b0:b0 + BL, :] = results[core]["at_out"]
    return (c, at)
